# revision 3
# baseline (speedup 1.0000x reference)
"""Single-head causal attention (B=4, T=4096, n_embd=1024, head=64) on 8 trn2 cores.

One SPMD program, 8 cores, one launch.  Core c -> batch b=c//2, half h=c%2.
Causal-balanced q-block (512 rows) assignment: half0 {0,3,4,7}, half1 {1,2,5,6}.

To keep the instruction stream identical across cores, each core runs 4 fixed
attention "slots" with k-ranges {8,16,24,32} k-blocks (128 keys each).  A slot
hosts one of the core's q-blocks (which one is per-core DATA, not control flow):
  half0: slots host qb {0,3,4,7} (own nk {4,16,20,32})
  half1: slots host qb {1,2,5,6} (own nk {8,12,24,28})
The last 8 k-blocks (4 pairs) of every slot get a mask multiply; per masked
pair one DVE scalar_tensor_tensor computes (dtab2 >= thr) * P with a fp16
column-minus-row table whose second half is pre-shifted by -128 (so one
threshold covers both k-blocks of the pair).  This zeroes both the causal
diagonal and the slot padding (own nk < slot nk).

Math (S^T formulation, bf16 inputs / fp32 PSUM):
  S^T[tk,tq] = K_blk^T.T @ Q^T          (PE bf16, psum [128, 2*512] fp32)
  P^T = exp(S^T / 8)  -> bf16           (one ACT op over both banks; no max-sub
                                         needed: S ~ N(0,1), exp can't overflow)
  P^T *= mask (last 4 pairs of slot)    (DVE STT, fp16 table)
  O_aug^T[65,512] += V_aug_blk.T @ P^T  (PE bf16; V_aug col 64 = ones => row 64
                                         of O_aug accumulates the softmax denom)
Epilogue per slot: PE-transpose O_aug^T -> [128tq, 65] fp32, reciprocal of
col 64, DVE scale -> natural [128,64] rows fp32, DMA out.  Host reassembles.
"""

import numpy as np
import ml_dtypes

BF16 = ml_dtypes.bfloat16

B, T, NE, HD = 4, 4096, 1024, 64
QB = 512            # q-block width
KB = 128            # k-block width
NQB = T // QB       # 8 t-blocks
NT = NE // 128      # 8 n-tiles (projection contraction)
SLOT_NK = [8, 16, 24, 32]          # k-blocks per slot (pairs: 4, 8, 12, 16)
HALF_QBS = [[0, 3, 4, 7], [1, 2, 5, 6]]   # slot si hosts q-block HALF_QBS[h][si]

_CACHE = {}


def _host_tables(half):
    """Per-core mask thresholds [16] (one per masked pair) and q-select
    offsets [4].

    Mask for slot si, masked pair m (k-blocks kx=nk-8+2m and kx+1):
    valid(i, c) iff qoff + c >= kx*128 + i  iff  (c - i) >= 128*kx - qoff.
    The pair's second k-block uses the same threshold against (c - i - 128).
    """
    thr = np.zeros(16, dtype=np.float16)
    qoffs = np.zeros(4, dtype=np.int32)
    for si, nk in enumerate(SLOT_NK):
        own_qb = HALF_QBS[half][si]
        qoffs[si] = own_qb * QB
        for m in range(4):
            kx = nk - 8 + 2 * m
            thr[si * 4 + m] = np.float16(128.0 * kx - float(qoffs[si]))
    return thr, qoffs


def _build_program():
    import concourse.bass as bass
    import concourse.mybir as mybir
    import concourse.tile as tile

    f32 = mybir.dt.float32
    f16 = mybir.dt.float16
    bf16 = mybir.dt.bfloat16
    i32 = mybir.dt.int32
    AF = mybir.ActivationFunctionType
    MS = bass.MemorySpace
    nc = bass.Bass("TRN2", target_bir_lowering=True, debug=False,
                   enable_asserts=False)

    xt_d = nc.dram_tensor("xt", [NE, T], bf16, kind="ExternalInput").ap()
    wkv_d = nc.dram_tensor("wkv", [NE, 128], bf16, kind="ExternalInput").ap()
    wq_d = nc.dram_tensor("wq", [NE, HD], bf16, kind="ExternalInput").ap()
    ident_d = nc.dram_tensor("ident", [128, 128], f32, kind="ExternalInput").ap()
    identh_d = nc.dram_tensor("identh", [128, 64], bf16, kind="ExternalInput").ap()
    dtab_d = nc.dram_tensor("dtab", [128, 2 * QB], f16, kind="ExternalInput").ap()
    thr_d = nc.dram_tensor("thr", [128, 16], f16, kind="ExternalInput").ap()
    qoffs_d = nc.dram_tensor("qoffs", [1, 4], i32, kind="ExternalInput").ap()
    out_d = nc.dram_tensor("out", [4 * QB, HD], f32, kind="ExternalOutput").ap()

    with tile.TileContext(nc) as tc:
        with (
            tc.tile_pool(name="consts", bufs=1) as cpool,
            tc.tile_pool(name="big", bufs=1) as bigpool,
            tc.tile_pool(name="xt", bufs=2) as xtpool,
            tc.tile_pool(name="pt", bufs=3) as ptpool,
            tc.tile_pool(name="osb", bufs=2) as osbpool,
            tc.tile_pool(name="onat", bufs=2) as onatpool,
            tc.tile_pool(name="rec", bufs=2) as recpool,
            tc.tile_pool(name="sps", bufs=2, space=MS.PSUM) as spool,
            tc.tile_pool(name="ops", bufs=2, space=MS.PSUM) as opool,
            tc.tile_pool(name="projps", bufs=2, space=MS.PSUM) as projpool,
        ):
            # ---- constants ----
            wkv_sb = cpool.tile([128, NT, 128], bf16)
            nc.gpsimd.dma_start(wkv_sb[:], wkv_d.rearrange("(nt p) m -> p nt m", p=128))
            wq_sb = cpool.tile([128, NT, HD], bf16)
            nc.gpsimd.dma_start(wq_sb[:], wq_d.rearrange("(nt p) m -> p nt m", p=128))
            ident = cpool.tile([128, 128], f32)
            nc.gpsimd.dma_start(ident[:], ident_d[:])
            identh = cpool.tile([128, 64], bf16)
            nc.gpsimd.dma_start(identh[:], identh_d[:])
            dtab = cpool.tile([128, 2 * QB], f16)
            nc.gpsimd.dma_start(dtab[:], dtab_d[:])
            thr = cpool.tile([128, 16], f16)
            nc.gpsimd.dma_start(thr[:], thr_d[:])
            qoffs = cpool.tile([1, 4], i32)
            nc.gpsimd.dma_start(qoffs[:], qoffs_d[:])

            # ---- persistent sbuf state ----
            kvt = bigpool.tile([128, T], bf16)         # 0:64 K^T, 64:128 V^T
            qt_all = bigpool.tile([64, T], bf16)       # Q^T all 8 panels
            qt_sel = bigpool.tile([64, 4 * QB], bf16)  # slot-ordered Q^T
            v_aug = bigpool.tile([128, 32 * 65], bf16) # V natural + ones col
            nc.vector.memset(v_aug[:], 1.0)

            def dyn_load(ap, lo, hi):
                tmp = nc.vector.alloc_register(f"dyn{nc.next_id()}")
                nc.vector.reg_load(tmp, ap)
                return nc.vector.snap(tmp, donate=True, min_val=lo, max_val=hi)

            def emit_attention(si):
                nk = SLOT_NK[si]
                npair = nk // 2
                o_ps = opool.tile([65, QB], f32, tag="ops")
                for p in range(npair):
                    ka, kb2 = 2 * p, 2 * p + 1
                    s_ps = spool.tile([128, 2 * QB], f32, tag="sps")
                    nc.tensor.matmul(
                        s_ps[:, 0:QB],
                        kvt[0:64, ka * KB:(ka + 1) * KB],
                        qt_sel[:, si * QB:(si + 1) * QB],
                        start=True, stop=True)
                    nc.tensor.matmul(
                        s_ps[:, QB:2 * QB],
                        kvt[0:64, kb2 * KB:(kb2 + 1) * KB],
                        qt_sel[:, si * QB:(si + 1) * QB],
                        start=True, stop=True)
                    pt = ptpool.tile([128, 2 * QB], bf16, tag="pt")
                    nc.scalar.activation(pt[:], s_ps[:], AF.Exp,
                                         scale=float(HD) ** -0.5)
                    m = p - (npair - 4)
                    if m >= 0:
                        # both k-blocks of the pair masked with one STT:
                        # pt = (dtab2 >= thr) * pt
                        nc.vector.scalar_tensor_tensor(
                            pt[:],
                            dtab[:],
                            thr[:, si * 4 + m: si * 4 + m + 1],
                            pt[:],
                            mybir.AluOpType.is_ge,
                            mybir.AluOpType.mult)
                    nc.tensor.matmul(
                        o_ps[:], v_aug[:, ka * 65:ka * 65 + 65], pt[:, 0:QB],
                        start=(p == 0), stop=False, skip_group_check=True)
                    nc.tensor.matmul(
                        o_ps[:], v_aug[:, kb2 * 65:kb2 * 65 + 65],
                        pt[:, QB:2 * QB],
                        start=False, stop=(p == npair - 1),
                        skip_group_check=True)
                # epilogue
                ot_sb = osbpool.tile([65, QB], f32, tag="osb")
                nc.vector.tensor_copy(ot_sb[:], o_ps[:])
                for u in range(QB // 128):
                    tp_ps = projpool.tile([128, QB], f32, tag="proj")
                    nc.tensor.transpose(
                        tp_ps[:, 0:65], ot_sb[:, u * 128:(u + 1) * 128],
                        ident[0:65, 0:65])
                    rec = recpool.tile([128, 1], f32, tag="rec")
                    nc.vector.reciprocal(rec[:], tp_ps[:, 64:65])
                    o_nat = onatpool.tile([128, HD], f32, tag="onat")
                    nc.vector.tensor_scalar(
                        o_nat[:], tp_ps[:, 0:HD], rec[:], None,
                        mybir.AluOpType.mult)
                    nc.sync.dma_start(
                        out_d[si * QB + u * 128: si * QB + (u + 1) * 128, :],
                        o_nat[:])

            # ---- main pipeline over t-blocks ----
            for tb in range(NQB):
                xt_sb = xtpool.tile([128, NT, QB], bf16, tag="xt")
                nc.gpsimd.dma_start(
                    xt_sb[:],
                    xt_d[:, tb * QB:(tb + 1) * QB].rearrange(
                        "(nt p) t -> p nt t", p=128))
                kv_ps = projpool.tile([128, QB], f32, tag="proj")
                for ni in range(NT):
                    nc.tensor.matmul(kv_ps[:], wkv_sb[:, ni, :], xt_sb[:, ni, :],
                                     start=(ni == 0), stop=(ni == NT - 1))
                nc.vector.tensor_copy(kvt[:, tb * QB:(tb + 1) * QB], kv_ps[:])
                q_ps = projpool.tile([64, QB], f32, tag="proj")
                for ni in range(NT):
                    nc.tensor.matmul(q_ps[:], wq_sb[:, ni, :], xt_sb[:, ni, :],
                                     start=(ni == 0), stop=(ni == NT - 1))
                nc.vector.tensor_copy(qt_all[:, tb * QB:(tb + 1) * QB], q_ps[:])
                for j in range(QB // KB):
                    kb = tb * (QB // KB) + j
                    tp_ps = projpool.tile([128, QB], bf16, tag="proj")
                    nc.tensor.transpose(
                        tp_ps[:, 0:64], kvt[64:128, kb * KB:(kb + 1) * KB],
                        identh[64:128, 0:64])
                    nc.vector.tensor_copy(v_aug[:, kb * 65:kb * 65 + 64],
                                          tp_ps[:, 0:64])
                if tb % 2 == 1:
                    si = tb // 2
                    with tc.tile_critical():
                        qoff = dyn_load(qoffs[0:1, si:si + 1], 0, T - QB)
                        nc.vector.tensor_copy(
                            qt_sel[:, si * QB:(si + 1) * QB],
                            qt_all[:, bass.ds(qoff, QB)])
                    emit_attention(si)

    _legalize_matmul_waits(nc)
    return nc


def _legalize_matmul_waits(nc):
    """walrus' LW template encodes at most one sync-wait; hoist extra waits
    from Matmult instructions onto a preceding PE NoOp (same queue, so
    ordering semantics are identical)."""
    import concourse.mybir as mybir

    for f in nc.m.functions:
        for bb in f.blocks:
            new_insts = []
            for inst in bb.instructions:
                si = inst.sync_info
                if (si is not None and si.on_wait and len(si.on_wait) >= 2):
                    for w in si.on_wait:
                        nop = mybir.InstNoOp(
                            name=nc.get_next_instruction_name(),
                            text_hint="wait_hoist", bass_nofuse=True)
                        nop.engine = inst.engine
                        nop.sync_info = mybir.SyncInfo(
                            on_wait=[w], on_update=[])
                        new_insts.append(nop)
                    inst.sync_info = mybir.SyncInfo(
                        on_wait=[], on_update=list(si.on_update or []))
                new_insts.append(inst)
            del bb.instructions[:]
            for i in new_insts:
                bb.instructions.append(i)


def _make_inputs(x, Wq, Wk, Wv):
    wkv = np.ascontiguousarray(
        np.concatenate([Wk, Wv], axis=1)).astype(BF16)
    wq = np.ascontiguousarray(np.asarray(Wq, dtype=np.float32)).astype(BF16)
    ident = np.eye(128, dtype=np.float32)
    identh = np.zeros((128, 64), dtype=np.float32)
    identh[64:128, :] = np.eye(64, dtype=np.float32)
    identh = identh.astype(BF16)
    col = np.arange(QB, dtype=np.float32)[None, :]
    row = np.arange(128, dtype=np.float32)[:, None]
    d = col - row
    dtab = np.concatenate([d, d - 128.0], axis=1).astype(np.float16)

    in_maps = []
    for c in range(8):
        b, half = c // 2, c % 2
        thr, qoffs = _host_tables(half)
        thr_rep = np.ascontiguousarray(np.tile(thr[None, :], (128, 1)))
        xt = np.ascontiguousarray(
            np.asarray(x[b], dtype=np.float32).T).astype(BF16)
        in_maps.append({
            "xt": xt, "wkv": wkv, "wq": wq, "ident": ident, "identh": identh,
            "dtab": dtab, "thr": thr_rep, "qoffs": qoffs.reshape(1, 4),
        })
    return in_maps


def kernel(x, Wq, Wk, Wv, _want_results=False, _trace=False):
    from concourse import bass_utils

    if "prog" not in _CACHE:
        _CACHE["prog"] = _build_program()
    nc = _CACHE["prog"]
    in_maps = _make_inputs(x, Wq, Wk, Wv)
    res = bass_utils.run_bass_kernel_spmd(nc, in_maps, core_ids=list(range(8)),
                                          trace=_trace)
    out = np.zeros((B, T, HD), dtype=np.float32)
    for c in range(8):
        b, half = c // 2, c % 2
        o = res.results[c]["out"]
        for si in range(4):
            qb = HALF_QBS[half][si]
            out[b, qb * QB:(qb + 1) * QB, :] = o[si * QB:(si + 1) * QB, :]
    if _want_results:
        return out, res
    return out


if __name__ == "__main__":
    # smoke test against a small numpy reference
    rng = np.random.default_rng(0)
    x = rng.standard_normal((B, T, NE), dtype=np.float32)
    s = 1.0 / np.sqrt(NE)
    Wq = rng.standard_normal((NE, HD), dtype=np.float32) * s
    Wk = rng.standard_normal((NE, HD), dtype=np.float32) * s
    Wv = rng.standard_normal((NE, HD), dtype=np.float32) * s
    out = kernel(x, Wq, Wk, Wv)
    q = x @ Wq
    k = x @ Wk
    v = x @ Wv
    aff = np.einsum('bth,bsh->bts', q, k) / np.sqrt(HD)
    mask = np.tril(np.ones((T, T), dtype=bool))
    aff = np.where(mask, aff, -np.inf)
    aff -= aff.max(axis=-1, keepdims=True)
    att = np.exp(aff)
    att /= att.sum(axis=-1, keepdims=True)
    ref = np.einsum('bts,bsh->bth', att, v)
    rel = np.linalg.norm(out - ref) / np.linalg.norm(ref)
    print("rel err:", rel)


# revision 5
# speedup vs baseline: 1.4651x; 1.4651x over previous
"""Single-head causal attention (B=4, T=4096, n_embd=1024, head=64) on 8 trn2 cores.

One SPMD program, 8 cores, one launch.  Core c -> batch b=c//2, half h=c%2.
Causal-balanced q-block (512 rows) assignment: half0 {0,3,4,7}, half1 {1,2,5,6}.

Uniform instruction stream across cores: each core runs 4 fixed attention
"slots" with k-ranges {8,16,24,32} k-blocks (128 keys each).  Which q-block a
slot hosts is per-core DATA: the host gathers the core's own 4 q-blocks into a
separate `xq` input (slot order), so there is no dynamic addressing on device.
The last 8 k-blocks (4 pairs) of every slot get a mask multiply; per masked
pair one DVE scalar_tensor_tensor computes (dtab2 >= thr) * P with an fp16
column-minus-row table whose second half is pre-shifted by -128 (one threshold
covers both k-blocks of a pair).  This zeroes both the causal diagonal and the
slot padding.

Math (S^T formulation, bf16 inputs / fp32 PSUM):
  S^T[tk,tq] = K_blk^T.T @ Q^T     (PE bf16; the two k-blocks of a pair run as
                                    row-tiled 64x128 matmuls on array rows
                                    0:63 / 64:127 concurrently -> 2 psum banks)
  P^T = exp(S^T / 8)  -> bf16      (one ACT op over both banks)
  P^T *= mask (last 4 pairs)       (DVE STT, fp16 table)
  O_aug^T[65,512] += V_aug_blk.T @ P^T  (PE bf16; V_aug col 64 = ones => row 64
                                    of O_aug accumulates the softmax denom)
Q projection runs per slot on xq as col-tiled M=64 matmul pairs that write the
same Q^T to psum partitions 0:63 and 64:127 (the row-tiled S needs Q^T on both
halves).  K^T is likewise replicated to partitions 64:127 via an SBUF->SBUF
DMA; attention for slot si is deferred to t-block 2si+2 to hide that latency.
Epilogue per slot: PE-transpose O_aug^T -> [128tq, 65] fp32, reciprocal of
col 64, DVE scale -> natural [128,64] rows fp32, DMA out.  Host reassembles.
"""

import numpy as np
import ml_dtypes

BF16 = ml_dtypes.bfloat16

B, T, NE, HD = 4, 4096, 1024, 64
QB = 512            # q-block width
KB = 128            # k-block width
NQB = T // QB       # 8 t-blocks
NT = NE // 128      # 8 n-tiles (projection contraction)
SLOT_NK = [8, 16, 24, 32]          # k-blocks per slot (pairs: 4, 8, 12, 16)
HALF_QBS = [[0, 3, 4, 7], [1, 2, 5, 6]]   # slot si hosts q-block HALF_QBS[h][si]

_CACHE = {}


def _host_thr(half):
    """Per-core mask thresholds [16] (one per masked pair).

    Mask for slot si, masked pair m (k-blocks kx=nk-8+2m and kx+1):
    valid(i, c) iff qoff + c >= kx*128 + i  iff  (c - i) >= 128*kx - qoff.
    The pair's second k-block uses the same threshold against (c - i - 128).
    """
    thr = np.zeros(16, dtype=np.float16)
    for si, nk in enumerate(SLOT_NK):
        qoff = HALF_QBS[half][si] * QB
        for m in range(4):
            kx = nk - 8 + 2 * m
            thr[si * 4 + m] = np.float16(128.0 * kx - float(qoff))
    return thr


def _build_program():
    import concourse.bass as bass
    import concourse.mybir as mybir
    import concourse.tile as tile

    f32 = mybir.dt.float32
    f16 = mybir.dt.float16
    bf16 = mybir.dt.bfloat16
    AF = mybir.ActivationFunctionType
    MS = bass.MemorySpace
    nc = bass.Bass("TRN2", target_bir_lowering=True, debug=False,
                   enable_asserts=False)

    xt_d = nc.dram_tensor("xt", [NE, T], bf16, kind="ExternalInput").ap()
    xq_d = nc.dram_tensor("xq", [NE, 4 * QB], bf16, kind="ExternalInput").ap()
    wkv_d = nc.dram_tensor("wkv", [NE, 128], bf16, kind="ExternalInput").ap()
    wq_d = nc.dram_tensor("wq", [NE, HD], bf16, kind="ExternalInput").ap()
    ident_d = nc.dram_tensor("ident", [128, 128], f32, kind="ExternalInput").ap()
    identh_d = nc.dram_tensor("identh", [128, 64], bf16, kind="ExternalInput").ap()
    dtab_d = nc.dram_tensor("dtab", [128, 2 * QB], f16, kind="ExternalInput").ap()
    thr_d = nc.dram_tensor("thr", [128, 16], f16, kind="ExternalInput").ap()
    out_d = nc.dram_tensor("out", [4 * QB, HD], f32, kind="ExternalOutput").ap()

    with tile.TileContext(nc) as tc:
        with (
            tc.tile_pool(name="consts", bufs=1) as cpool,
            tc.tile_pool(name="big", bufs=1) as bigpool,
            tc.tile_pool(name="xt", bufs=2) as xtpool,
            tc.tile_pool(name="xq", bufs=2) as xqpool,
            tc.tile_pool(name="pt", bufs=3) as ptpool,
            tc.tile_pool(name="osb", bufs=2) as osbpool,
            tc.tile_pool(name="onat", bufs=2) as onatpool,
            tc.tile_pool(name="rec", bufs=2) as recpool,
            tc.tile_pool(name="sps", bufs=2, space=MS.PSUM) as spool,
            tc.tile_pool(name="ops", bufs=2, space=MS.PSUM) as opool,
            tc.tile_pool(name="projps", bufs=2, space=MS.PSUM) as projpool,
        ):
            # ---- constants ----
            wkv_sb = cpool.tile([128, NT, 128], bf16)
            nc.gpsimd.dma_start(wkv_sb[:], wkv_d.rearrange("(nt p) m -> p nt m", p=128))
            wq_sb = cpool.tile([128, NT, HD], bf16)
            nc.gpsimd.dma_start(wq_sb[:], wq_d.rearrange("(nt p) m -> p nt m", p=128))
            ident = cpool.tile([128, 128], f32)
            nc.gpsimd.dma_start(ident[:], ident_d[:])
            identh = cpool.tile([128, 64], bf16)
            nc.gpsimd.dma_start(identh[:], identh_d[:])
            dtab = cpool.tile([128, 2 * QB], f16)
            nc.gpsimd.dma_start(dtab[:], dtab_d[:])
            thr = cpool.tile([128, 16], f16)
            nc.gpsimd.dma_start(thr[:], thr_d[:])

            # ---- persistent sbuf state ----
            kvt = bigpool.tile([128, T], bf16)         # 0:64 K^T, 64:128 V^T
            ktr = bigpool.tile([128, T], bf16)         # 64:128 = K^T replica
            qt_sel = bigpool.tile([128, 4 * QB], bf16) # own Q^T, both halves
            v_aug = bigpool.tile([128, 32 * 65], bf16) # V natural + ones col
            nc.vector.memset(v_aug[:], 1.0)

            def emit_attention(si):
                nk = SLOT_NK[si]
                npair = nk // 2
                qa = qt_sel[0:64, si * QB:(si + 1) * QB]
                qb = qt_sel[64:128, si * QB:(si + 1) * QB]
                o_ps = opool.tile([65, QB], f32, tag="ops")
                for p in range(npair):
                    ka, kb2 = 2 * p, 2 * p + 1
                    s_ps = spool.tile([128, 2 * QB], f32, tag="sps")
                    nc.tensor.matmul(
                        s_ps[:, 0:QB],
                        kvt[0:64, ka * KB:(ka + 1) * KB],
                        qa, start=True, stop=True)
                    nc.tensor.matmul(
                        s_ps[:, QB:2 * QB],
                        ktr[64:128, kb2 * KB:(kb2 + 1) * KB],
                        qb, start=True, stop=True)
                    pt = ptpool.tile([128, 2 * QB], bf16, tag="pt")
                    nc.scalar.activation(pt[:], s_ps[:], AF.Exp,
                                         scale=float(HD) ** -0.5)
                    m = p - (npair - 4)
                    if m >= 0:
                        # both k-blocks of the pair: pt = (dtab2 >= thr) * pt
                        nc.vector.scalar_tensor_tensor(
                            pt[:], dtab[:],
                            thr[:, si * 4 + m: si * 4 + m + 1],
                            pt[:],
                            mybir.AluOpType.is_ge,
                            mybir.AluOpType.mult)
                    nc.tensor.matmul(
                        o_ps[:], v_aug[:, ka * 65:ka * 65 + 65], pt[:, 0:QB],
                        start=(p == 0), stop=False, skip_group_check=True)
                    nc.tensor.matmul(
                        o_ps[:], v_aug[:, kb2 * 65:kb2 * 65 + 65],
                        pt[:, QB:2 * QB],
                        start=False, stop=(p == npair - 1),
                        skip_group_check=True)
                # epilogue
                ot_sb = osbpool.tile([65, QB], f32, tag="osb")
                nc.vector.tensor_copy(ot_sb[:], o_ps[:])
                for u in range(QB // 128):
                    tp_ps = projpool.tile([128, QB], f32, tag="proj")
                    nc.tensor.transpose(
                        tp_ps[:, 0:65], ot_sb[:, u * 128:(u + 1) * 128],
                        ident[0:65, 0:65])
                    rec = recpool.tile([128, 1], f32, tag="rec")
                    nc.vector.reciprocal(rec[:], tp_ps[:, 64:65])
                    o_nat = onatpool.tile([128, HD], f32, tag="onat")
                    nc.vector.tensor_scalar(
                        o_nat[:], tp_ps[:, 0:HD], rec[:], None,
                        mybir.AluOpType.mult)
                    nc.sync.dma_start(
                        out_d[si * QB + u * 128: si * QB + (u + 1) * 128, :],
                        o_nat[:])

            # ---- main pipeline over t-blocks ----
            for tb in range(NQB):
                xt_sb = xtpool.tile([128, NT, QB], bf16, tag="xt")
                src = xt_d[:, tb * QB:(tb + 1) * QB].rearrange(
                    "(nt p) t -> p nt t", p=128)
                if tb == 0:
                    # split so the first projection matmul starts sooner
                    nc.gpsimd.dma_start(xt_sb[:, 0:2, :], src[:, 0:2, :])
                    nc.gpsimd.dma_start(xt_sb[:, 2:NT, :], src[:, 2:NT, :])
                else:
                    nc.gpsimd.dma_start(xt_sb[:], src)
                kv_ps = projpool.tile([128, QB], f32, tag="proj")
                for ni in range(NT):
                    nc.tensor.matmul(kv_ps[:], wkv_sb[:, ni, :], xt_sb[:, ni, :],
                                     start=(ni == 0), stop=(ni == NT - 1))
                nc.vector.tensor_copy(kvt[:, tb * QB:(tb + 1) * QB], kv_ps[:])
                # replicate K^T to partitions 64:127 for the row-tiled S matmul
                nc.sync.dma_start(ktr[64:128, tb * QB:(tb + 1) * QB],
                                  kvt[0:64, tb * QB:(tb + 1) * QB])
                for j in range(QB // KB):
                    kb = tb * (QB // KB) + j
                    tp_ps = projpool.tile([128, QB], bf16, tag="proj")
                    nc.tensor.transpose(
                        tp_ps[:, 0:64], kvt[64:128, kb * KB:(kb + 1) * KB],
                        identh[64:128, 0:64])
                    nc.vector.tensor_copy(v_aug[:, kb * 65:kb * 65 + 64],
                                          tp_ps[:, 0:64])
                if tb % 2 == 1:
                    si = tb // 2
                    xq_sb = xqpool.tile([128, NT, QB], bf16, tag="xq")
                    nc.gpsimd.dma_start(
                        xq_sb[:],
                        xq_d[:, si * QB:(si + 1) * QB].rearrange(
                            "(nt p) t -> p nt t", p=128))
                    # Both col-tiled halves accumulate in ONE psum bank: a
                    # start=True clears has_written for the WHOLE bank, so
                    # only the very first matmul may carry it (later matmuls
                    # overwrite where the bit is clear, accumulate where set).
                    q2_ps = projpool.tile([128, QB], f32, tag="proj")
                    for ni in range(NT):
                        nc.tensor.matmul(q2_ps[0:64, :], wq_sb[:, ni, :],
                                         xq_sb[:, ni, :],
                                         start=(ni == 0), stop=False,
                                         skip_group_check=True)
                        nc.tensor.matmul(q2_ps[64:128, :], wq_sb[:, ni, :],
                                         xq_sb[:, ni, :],
                                         start=False, stop=(ni == NT - 1),
                                         skip_group_check=True)
                    nc.vector.tensor_copy(qt_sel[:, si * QB:(si + 1) * QB],
                                          q2_ps[:])
                if tb >= 2 and tb % 2 == 0:
                    emit_attention(tb // 2 - 1)
            emit_attention(3)

    _legalize_matmul_waits(nc)
    return nc


def _legalize_matmul_waits(nc):
    """walrus' LW template encodes at most one sync-wait; hoist extra waits
    from Matmult instructions onto a preceding PE NoOp (same queue, so
    ordering semantics are identical)."""
    import concourse.mybir as mybir

    for f in nc.m.functions:
        for bb in f.blocks:
            new_insts = []
            for inst in bb.instructions:
                si = inst.sync_info
                if (si is not None and si.on_wait and len(si.on_wait) >= 2):
                    for w in si.on_wait:
                        nop = mybir.InstNoOp(
                            name=nc.get_next_instruction_name(),
                            text_hint="wait_hoist", bass_nofuse=True)
                        nop.engine = inst.engine
                        nop.sync_info = mybir.SyncInfo(
                            on_wait=[w], on_update=[])
                        new_insts.append(nop)
                    inst.sync_info = mybir.SyncInfo(
                        on_wait=[], on_update=list(si.on_update or []))
                new_insts.append(inst)
            del bb.instructions[:]
            for i in new_insts:
                bb.instructions.append(i)


def _make_inputs(x, Wq, Wk, Wv):
    wkv = np.ascontiguousarray(
        np.concatenate([Wk, Wv], axis=1)).astype(BF16)
    wq = np.ascontiguousarray(np.asarray(Wq, dtype=np.float32)).astype(BF16)
    ident = np.eye(128, dtype=np.float32)
    identh = np.zeros((128, 64), dtype=np.float32)
    identh[64:128, :] = np.eye(64, dtype=np.float32)
    identh = identh.astype(BF16)
    col = np.arange(QB, dtype=np.float32)[None, :]
    row = np.arange(128, dtype=np.float32)[:, None]
    d = col - row
    dtab = np.concatenate([d, d - 128.0], axis=1).astype(np.float16)

    in_maps = []
    for c in range(8):
        b, half = c // 2, c % 2
        thr = _host_thr(half)
        thr_rep = np.ascontiguousarray(np.tile(thr[None, :], (128, 1)))
        xb = np.asarray(x[b], dtype=np.float32)
        xt = np.ascontiguousarray(xb.T).astype(BF16)
        xq = np.concatenate(
            [xb[qb * QB:(qb + 1) * QB, :].T for qb in HALF_QBS[half]],
            axis=1)
        xq = np.ascontiguousarray(xq).astype(BF16)
        in_maps.append({
            "xt": xt, "xq": xq, "wkv": wkv, "wq": wq, "ident": ident,
            "identh": identh, "dtab": dtab, "thr": thr_rep,
        })
    return in_maps


def kernel(x, Wq, Wk, Wv, _want_results=False, _trace=False):
    from concourse import bass_utils

    if "prog" not in _CACHE:
        _CACHE["prog"] = _build_program()
    nc = _CACHE["prog"]
    in_maps = _make_inputs(x, Wq, Wk, Wv)
    res = bass_utils.run_bass_kernel_spmd(nc, in_maps, core_ids=list(range(8)),
                                          trace=_trace)
    out = np.zeros((B, T, HD), dtype=np.float32)
    for c in range(8):
        b, half = c // 2, c % 2
        o = res.results[c]["out"]
        for si in range(4):
            qb = HALF_QBS[half][si]
            out[b, qb * QB:(qb + 1) * QB, :] = o[si * QB:(si + 1) * QB, :]
    if _want_results:
        return out, res
    return out


# revision 8
# speedup vs baseline: 1.7973x; 1.2267x over previous
"""Single-head causal attention (B=4, T=4096, n_embd=1024, head=64) on 8 trn2 cores.

One SPMD program, 8 cores, one launch.  Core c -> batch b=c//2, half h=c%2.
Causal-balanced q-block (512 rows) assignment: half0 {0,3,4,7}, half1 {1,2,5,6}.

Uniform instruction stream across cores; everything core-specific is DATA:
  - xq: the core's own 4 q-blocks of x^T, host-gathered in slot order
  - masks: 16 precomputed [128,1024] 0/1 bf16 tiles (slot si, masked pair m)
Slot si covers SLOT_NK[si] = {8,16,24,32} k-blocks (128 keys each); the last
4 pairs of each slot are mask-multiplied (covers both the causal diagonal and
the padding when the hosted q-block needs fewer k-blocks than the slot).

Math (S^T formulation, bf16 inputs / fp32 PSUM):
  S^T[tk,tq] = K_blk^T.T @ Q^T   (the 2 k-blocks of a pair run as row-tiled
                                  64x128 matmuls on array rows 0:63 / 64:127
                                  concurrently -> 2 psum banks)
  P^T = exp(S^T / 8) -> bf16     (one ACT op over both banks)
  P^T *= mask                    (DVE tensor_tensor, bf16 2x mode)
  O_aug^T[65,512] += V_aug_blk.T @ P^T   (V_aug col 64 = ones => row 64 of
                                  O_aug accumulates the softmax denominator)

Schedule (all Q projected up front; attention pairs emitted incrementally one
t-block after their keys are projected, so the PE never idles long enough for
the HAM clock gate to re-throttle, and the tail holds only 2 pairs):
  tb0: Q(slots 0-3) col-tiled to both psum halves; KV proj(0)
  tb t: KV proj(t) + V transposes + pairs: slot0@tb1-2, slot1@tb3-4,
        slot2@tb5-6, slot3 2 pairs/tb; epilogues at slot completion
K^T is replicated to partitions 64:127 (SBUF->SBUF DMA, one t-block of slack)
for the row-tiled S; Q^T is written to both halves by a col-tiled matmul pair.
Epilogue per slot: PE-transpose O_aug^T -> [128tq,65] fp32, reciprocal of col
64, scale -> natural [128,64] fp32 rows, DMA out.  Host reassembles slots.
"""

import numpy as np
import ml_dtypes

BF16 = ml_dtypes.bfloat16

B, T, NE, HD = 4, 4096, 1024, 64
QB = 512            # q-block width
KB = 128            # k-block width
NQB = T // QB       # 8 t-blocks
NT = NE // 128      # 8 n-tiles (projection contraction)
SLOT_NK = [8, 16, 24, 32]          # k-blocks per slot (pairs: 4, 8, 12, 16)
HALF_QBS = [[0, 3, 4, 7], [1, 2, 5, 6]]   # slot si hosts q-block HALF_QBS[h][si]

# pair emission schedule: _SCHED[tb] = [(si, p), ...] emitted after proj(tb)
_SCHED = {tb: [] for tb in range(1, NQB)}
for _tb in range(1, 3):
    _SCHED[_tb] += [(0, p) for p in range(2 * (_tb - 1), 2 * _tb)]
for _tb in range(3, 5):
    _SCHED[_tb] += [(1, p) for p in range(4 * (_tb - 3), 4 * (_tb - 2))]
for _tb in range(5, 7):
    _SCHED[_tb] += [(2, p) for p in range(6 * (_tb - 5), 6 * (_tb - 4))]
for _tb in range(1, 8):
    _SCHED[_tb] += [(3, p) for p in range(2 * (_tb - 1), 2 * _tb)]
_POST = [(3, 14), (3, 15)]
_EPI_AT = {2: 0, 4: 1, 6: 2}   # epilogue of slot si after tb's pairs

_CACHE = {}


def _build_program():
    import concourse.bass as bass
    import concourse.mybir as mybir
    import concourse.tile as tile

    f32 = mybir.dt.float32
    bf16 = mybir.dt.bfloat16
    AF = mybir.ActivationFunctionType
    MS = bass.MemorySpace
    nc = bass.Bass("TRN2", target_bir_lowering=True, debug=False,
                   enable_asserts=False)

    xt_d = nc.dram_tensor("xt", [NE, T], bf16, kind="ExternalInput").ap()
    xq_d = nc.dram_tensor("xq", [NE, 4 * QB], bf16, kind="ExternalInput").ap()
    wkv_d = nc.dram_tensor("wkv", [NE, 128], bf16, kind="ExternalInput").ap()
    wq_d = nc.dram_tensor("wq", [NE, HD], bf16, kind="ExternalInput").ap()
    ident_d = nc.dram_tensor("ident", [128, 128], f32, kind="ExternalInput").ap()
    identh_d = nc.dram_tensor("identh", [128, 64], bf16, kind="ExternalInput").ap()
    masks_d = nc.dram_tensor("masks", [128, 16 * 2 * QB], bf16,
                             kind="ExternalInput").ap()
    out_d = nc.dram_tensor("out", [4 * QB, HD], f32, kind="ExternalOutput").ap()

    with tile.TileContext(nc) as tc:
        with (
            tc.tile_pool(name="consts", bufs=1) as cpool,
            tc.tile_pool(name="big", bufs=1) as bigpool,
            tc.tile_pool(name="xt", bufs=3) as xtpool,
            tc.tile_pool(name="xq", bufs=2) as xqpool,
            tc.tile_pool(name="pt", bufs=3) as ptpool,
            tc.tile_pool(name="osb", bufs=4) as osbpool,
            tc.tile_pool(name="onat", bufs=6) as onatpool,
            tc.tile_pool(name="rec", bufs=6) as recpool,
            tc.tile_pool(name="sps", bufs=2, space=MS.PSUM) as spool,
            tc.tile_pool(name="o3ps", bufs=1, space=MS.PSUM) as o3pool,
            tc.tile_pool(name="ops", bufs=1, space=MS.PSUM) as opool,
            tc.tile_pool(name="projps", bufs=1, space=MS.PSUM) as projpool,
            tc.tile_pool(name="vtps", bufs=1, space=MS.PSUM) as vtpool,
        ):
            # ---- constants (sync queue; gpsimd queue is for the big loads) ----
            wkv_sb = cpool.tile([128, NT, 128], bf16)
            nc.sync.dma_start(wkv_sb[:], wkv_d.rearrange("(nt p) m -> p nt m", p=128))
            wq_sb = cpool.tile([128, NT, HD], bf16)
            nc.sync.dma_start(wq_sb[:], wq_d.rearrange("(nt p) m -> p nt m", p=128))
            ident = cpool.tile([128, 128], f32)
            nc.sync.dma_start(ident[:], ident_d[:])
            identh = cpool.tile([128, 64], bf16)
            nc.sync.dma_start(identh[:], identh_d[:])
            masks = cpool.tile([128, 16 * 2 * QB], bf16)
            nc.sync.dma_start(masks[:], masks_d[:])

            # ---- persistent sbuf state ----
            kvt = bigpool.tile([128, T], bf16)         # 0:64 K^T, 64:128 V^T
            ktr = bigpool.tile([128, T], bf16)         # 64:128 = K^T replica
            qt_sel = bigpool.tile([128, 4 * QB], bf16) # own Q^T, both halves
            v_aug = bigpool.tile([128, 32 * 65], bf16) # V natural + ones col
            nc.vector.memset(v_aug[:], 1.0)

            o_ps_of = {}

            def emit_pair(si, p):
                npair = SLOT_NK[si] // 2
                if p == 0:
                    pool = o3pool if si == 3 else opool
                    o_ps_of[si] = pool.tile(
                        [65, QB], f32, name=f"o_ps{si}",
                        tag="o3" if si == 3 else "ops")
                o_ps = o_ps_of[si]
                ka, kb2 = 2 * p, 2 * p + 1
                s_ps = spool.tile([128, 2 * QB], f32, tag="sps")
                nc.tensor.matmul(
                    s_ps[:, 0:QB],
                    kvt[0:64, ka * KB:(ka + 1) * KB],
                    qt_sel[0:64, si * QB:(si + 1) * QB],
                    start=True, stop=True)
                nc.tensor.matmul(
                    s_ps[:, QB:2 * QB],
                    ktr[64:128, kb2 * KB:(kb2 + 1) * KB],
                    qt_sel[64:128, si * QB:(si + 1) * QB],
                    start=True, stop=True)
                pt = ptpool.tile([128, 2 * QB], bf16, tag="pt")
                nc.scalar.activation(pt[:], s_ps[:], AF.Exp,
                                     scale=float(HD) ** -0.5)
                m = p - (npair - 4)
                if m >= 0:
                    idx = (si * 4 + m) * 2 * QB
                    nc.vector.tensor_tensor(
                        pt[:], pt[:], masks[:, idx:idx + 2 * QB],
                        mybir.AluOpType.mult)
                nc.tensor.matmul(
                    o_ps[:], v_aug[:, ka * 65:ka * 65 + 65], pt[:, 0:QB],
                    start=(p == 0), stop=False, skip_group_check=True)
                nc.tensor.matmul(
                    o_ps[:], v_aug[:, kb2 * 65:kb2 * 65 + 65],
                    pt[:, QB:2 * QB],
                    start=False, stop=(p == npair - 1),
                    skip_group_check=True)

            def emit_epilogue(si):
                o_ps = o_ps_of[si]
                ot_sb = osbpool.tile([65, QB], f32, tag="osb")
                nc.any.tensor_copy(ot_sb[:], o_ps[:])
                for u in range(QB // 128):
                    tp_ps = vtpool.tile([128, QB], f32, tag="vt")
                    nc.tensor.transpose(
                        tp_ps[:, 0:65], ot_sb[:, u * 128:(u + 1) * 128],
                        ident[0:65, 0:65])
                    rec = recpool.tile([128, 1], f32, tag="rec")
                    nc.vector.reciprocal(rec[:], tp_ps[:, 64:65])
                    o_nat = onatpool.tile([128, HD], f32, tag="onat")
                    nc.vector.tensor_scalar(
                        o_nat[:], tp_ps[:, 0:HD], rec[:], None,
                        mybir.AluOpType.mult)
                    nc.sync.dma_start(
                        out_d[si * QB + u * 128: si * QB + (u + 1) * 128, :],
                        o_nat[:])

            # ---- up-front Q projections (only need wq + xq) ----
            for si in range(4):
                xq_sb = xqpool.tile([128, NT, QB], bf16, tag="xq")
                nc.gpsimd.dma_start(
                    xq_sb[:],
                    xq_d[:, si * QB:(si + 1) * QB].rearrange(
                        "(nt p) t -> p nt t", p=128))
                # col-tiled halves write DIAGONAL slices of a 2-bank tile so
                # each accumulation group owns its psum bank outright
                q2_ps = spool.tile([128, 2 * QB], f32, tag="sps")
                for ni in range(NT):
                    nc.tensor.matmul(q2_ps[0:64, 0:QB], wq_sb[:, ni, :],
                                     xq_sb[:, ni, :],
                                     start=(ni == 0), stop=(ni == NT - 1))
                    nc.tensor.matmul(q2_ps[64:128, QB:2 * QB], wq_sb[:, ni, :],
                                     xq_sb[:, ni, :],
                                     start=(ni == 0), stop=(ni == NT - 1))
                nc.any.tensor_copy(qt_sel[0:64, si * QB:(si + 1) * QB],
                                   q2_ps[0:64, 0:QB])
                nc.any.tensor_copy(qt_sel[64:128, si * QB:(si + 1) * QB],
                                   q2_ps[64:128, QB:2 * QB])

            # ---- main pipeline over t-blocks ----
            for tb in range(NQB):
                xt_sb = xtpool.tile([128, NT, QB], bf16, tag="xt")
                src = xt_d[:, tb * QB:(tb + 1) * QB].rearrange(
                    "(nt p) t -> p nt t", p=128)
                if tb < 2:
                    # split so the first projection matmuls start sooner
                    for c in range(0, NT, 2):
                        nc.gpsimd.dma_start(xt_sb[:, c:c + 2, :],
                                            src[:, c:c + 2, :])
                else:
                    nc.gpsimd.dma_start(xt_sb[:], src)
                kv_ps = projpool.tile([128, QB], f32, tag="proj")
                for ni in range(NT):
                    nc.tensor.matmul(kv_ps[:], wkv_sb[:, ni, :], xt_sb[:, ni, :],
                                     start=(ni == 0), stop=(ni == NT - 1))
                nc.any.tensor_copy(kvt[:, tb * QB:(tb + 1) * QB], kv_ps[:])
                # replicate K^T to partitions 64:127 for the row-tiled S
                nc.sync.dma_start(ktr[64:128, tb * QB:(tb + 1) * QB],
                                  kvt[0:64, tb * QB:(tb + 1) * QB])
                for j in range(QB // KB):
                    kb = tb * (QB // KB) + j
                    tp_ps = vtpool.tile([128, QB], bf16, tag="vt")
                    nc.tensor.transpose(
                        tp_ps[:, 0:64], kvt[64:128, kb * KB:(kb + 1) * KB],
                        identh[64:128, 0:64])
                    nc.any.tensor_copy(v_aug[:, kb * 65:kb * 65 + 64],
                                       tp_ps[:, 0:64])
                for si, p in _SCHED.get(tb, []):
                    emit_pair(si, p)
                if tb in _EPI_AT:
                    emit_epilogue(_EPI_AT[tb])
            for si, p in _POST:
                emit_pair(si, p)
            emit_epilogue(3)

    _legalize_matmul_waits(nc)
    return nc


def _legalize_matmul_waits(nc):
    """walrus' LW template encodes at most one sync-wait; hoist extra waits
    from Matmult instructions onto a preceding PE NoOp (same queue, so
    ordering semantics are identical)."""
    import concourse.mybir as mybir

    for f in nc.m.functions:
        for bb in f.blocks:
            new_insts = []
            for inst in bb.instructions:
                si = inst.sync_info
                if (si is not None and si.on_wait and len(si.on_wait) >= 2):
                    for w in si.on_wait:
                        nop = mybir.InstNoOp(
                            name=nc.get_next_instruction_name(),
                            text_hint="wait_hoist", bass_nofuse=True)
                        nop.engine = inst.engine
                        nop.sync_info = mybir.SyncInfo(
                            on_wait=[w], on_update=[])
                        new_insts.append(nop)
                    inst.sync_info = mybir.SyncInfo(
                        on_wait=[], on_update=list(si.on_update or []))
                new_insts.append(inst)
            del bb.instructions[:]
            for i in new_insts:
                bb.instructions.append(i)


def _host_masks(half):
    """16 mask tiles [128, 1024] bf16: slot si, masked pair m covers k-blocks
    kx = nk-8+2m (cols 0:512) and kx+1 (cols 512:1024).
    valid(i, c) iff qoff + c >= kx*128 + i."""
    i = np.arange(128, dtype=np.int32)[:, None]
    c = np.arange(QB, dtype=np.int32)[None, :]
    tiles = []
    for si, nk in enumerate(SLOT_NK):
        qoff = HALF_QBS[half][si] * QB
        for m in range(4):
            for kx in (nk - 8 + 2 * m, nk - 8 + 2 * m + 1):
                tiles.append((qoff + c - kx * 128 - i) >= 0)
    return np.ascontiguousarray(
        np.concatenate(tiles, axis=1).astype(BF16))


def _make_inputs(x, Wq, Wk, Wv):
    wkv = np.ascontiguousarray(
        np.concatenate([Wk, Wv], axis=1)).astype(BF16)
    wq = np.ascontiguousarray(np.asarray(Wq, dtype=np.float32)).astype(BF16)
    ident = np.eye(128, dtype=np.float32)
    identh = np.zeros((128, 64), dtype=np.float32)
    identh[64:128, :] = np.eye(64, dtype=np.float32)
    identh = identh.astype(BF16)

    in_maps = []
    for c in range(8):
        b, half = c // 2, c % 2
        xb = np.asarray(x[b], dtype=np.float32)
        xt = np.ascontiguousarray(xb.T).astype(BF16)
        xq = np.concatenate(
            [xb[qb * QB:(qb + 1) * QB, :].T for qb in HALF_QBS[half]],
            axis=1)
        xq = np.ascontiguousarray(xq).astype(BF16)
        in_maps.append({
            "xt": xt, "xq": xq, "wkv": wkv, "wq": wq, "ident": ident,
            "identh": identh, "masks": _host_masks(half),
        })
    return in_maps


def kernel(x, Wq, Wk, Wv, _want_results=False, _trace=False):
    from concourse import bass_utils

    if "prog" not in _CACHE:
        _CACHE["prog"] = _build_program()
    nc = _CACHE["prog"]
    in_maps = _make_inputs(x, Wq, Wk, Wv)
    res = bass_utils.run_bass_kernel_spmd(nc, in_maps, core_ids=list(range(8)),
                                          trace=_trace)
    out = np.zeros((B, T, HD), dtype=np.float32)
    for c in range(8):
        b, half = c // 2, c % 2
        o = res.results[c]["out"]
        for si in range(4):
            qb = HALF_QBS[half][si]
            out[b, qb * QB:(qb + 1) * QB, :] = o[si * QB:(si + 1) * QB, :]
    if _want_results:
        return out, res
    return out


# revision 19
# speedup vs baseline: 1.8306x; 1.0185x over previous
"""Single-head causal attention (B=4, T=4096, n_embd=1024, head=64) on 8 trn2 cores.

One SPMD program, 8 cores, one launch.  Core c -> batch b=c//2, half h=c%2.
Causal-balanced q-block (512 rows) assignment: half0 {0,3,4,7}, half1 {1,2,5,6}.

Uniform instruction stream across cores; everything core-specific is DATA:
  - xq: the core's own 4 q-blocks of x^T, host-gathered in slot order
  - masks: 16 precomputed [128,1024] 0/1 bf16 tiles (slot si, masked pair m)
All device inputs are host-pre-swizzled into the exact SBUF layout (partition-
major, contiguous free dim) so every input DMA is a plain 2D copy with 8KB
contiguous runs per partition (the naive 3D gather moved only ~1KB per packet
and crawled at ~90 GB/s).

Slot si covers SLOT_NK[si] = {8,16,24,32} k-blocks (128 keys each); the last
4 pairs of each slot are mask-multiplied (covers both the causal diagonal and
the padding when the hosted q-block needs fewer k-blocks than the slot).

Math (S^T formulation, bf16 inputs / fp32 PSUM):
  S^T[tk,tq] = K_blk^T.T @ Q^T   (the 2 k-blocks of a pair run as row-tiled
                                  64x128 matmuls on array rows 0:63 / 64:127
                                  concurrently -> 2 psum banks)
  P^T = exp(S^T / 8) -> bf16     (one ACT op over both banks)
  P^T *= mask                    (DVE tensor_tensor, bf16 2x mode)
  O_aug^T[65,512] += V_aug_blk.T @ P^T   (V_aug col 64 = ones => row 64 of
                                  O_aug accumulates the softmax denominator)

Schedule: all Q projected up front (col-tiled M=64 pairs replicate Q^T to both
partition halves for the row-tiled S); attention pairs are emitted
incrementally one t-block after their keys are projected (slot0@tb1-2,
slot1@tb3-4, slot2@tb5-6, slot3 2 pairs/tb) and SOFTWARE-PIPELINED: the PE
stream is S(p+1) ... PV(p), so the next pair's S runs during exp(p) and the
ACT engine (the steady-state rate limiter) never waits on the serial
exp->mask->PV->S chain.  K^T is replicated to partitions 64:127 via an
SBUF->SBUF DMA with one t-block of slack.
Epilogue per slot (right after its last PV): PE-transpose O_aug^T ->
[128tq,65] fp32, reciprocal of col 64, scale -> natural [128,64] fp32 rows,
DMA out.  Host reassembles slots.
"""

import numpy as np
import ml_dtypes

BF16 = ml_dtypes.bfloat16

B, T, NE, HD = 4, 4096, 1024, 64
QB = 512            # q-block width
KB = 128            # k-block width
NQB = T // QB       # 8 t-blocks
NT = NE // 128      # 8 n-tiles (projection contraction)
SLOT_NK = [8, 16, 24, 32]          # k-blocks per slot (pairs: 4, 8, 12, 16)
HALF_QBS = [[0, 3, 4, 7], [1, 2, 5, 6]]   # slot si hosts q-block HALF_QBS[h][si]

# pair emission schedule: _SCHED[tb] = [(si, p), ...] emitted after proj(tb)
_SCHED = {tb: [] for tb in range(1, NQB)}
for _tb in range(1, 3):
    _SCHED[_tb] += [(0, p) for p in range(2 * (_tb - 1), 2 * _tb)]
for _tb in range(3, 5):
    _SCHED[_tb] += [(1, p) for p in range(4 * (_tb - 3), 4 * (_tb - 2))]
for _tb in range(5, 7):
    _SCHED[_tb] += [(2, p) for p in range(6 * (_tb - 5), 6 * (_tb - 4))]
for _tb in range(1, 8):
    _SCHED[_tb] += [(3, p) for p in range(2 * (_tb - 1), 2 * _tb)]
_POST = [(3, 14), (3, 15)]

_CACHE = {}


def _build_program():
    import concourse.bass as bass
    import concourse.mybir as mybir
    import concourse.tile as tile

    f32 = mybir.dt.float32
    bf16 = mybir.dt.bfloat16
    AF = mybir.ActivationFunctionType
    MS = bass.MemorySpace
    nc = bass.Bass("TRN2", target_bir_lowering=True, debug=False,
                   enable_asserts=False)

    # all pre-swizzled to [128 partitions, contiguous free]
    xt_d = nc.dram_tensor("xt", [128, NQB * NT * QB], bf16,
                          kind="ExternalInput").ap()
    xq_d = nc.dram_tensor("xq", [128, 4 * NT * QB], bf16,
                          kind="ExternalInput").ap()
    wkv_d = nc.dram_tensor("wkv", [128, NT * 128], bf16,
                           kind="ExternalInput").ap()
    wq_d = nc.dram_tensor("wq", [128, NT * HD], bf16,
                          kind="ExternalInput").ap()
    ident_d = nc.dram_tensor("ident", [128, 128], f32, kind="ExternalInput").ap()
    identh_d = nc.dram_tensor("identh", [128, 64], bf16, kind="ExternalInput").ap()
    masks_d = nc.dram_tensor("masks", [128, 16 * 2 * QB], bf16,
                             kind="ExternalInput").ap()
    out_d = nc.dram_tensor("out", [4 * QB, HD], f32, kind="ExternalOutput").ap()

    with tile.TileContext(nc) as tc:
        with (
            tc.tile_pool(name="consts", bufs=1) as cpool,
            tc.tile_pool(name="big", bufs=1) as bigpool,
            tc.tile_pool(name="xt", bufs=3) as xtpool,
            tc.tile_pool(name="xq", bufs=4) as xqpool,
            tc.tile_pool(name="pt", bufs=3) as ptpool,
            tc.tile_pool(name="osb", bufs=4) as osbpool,
            tc.tile_pool(name="onat", bufs=6) as onatpool,
            tc.tile_pool(name="rec", bufs=6) as recpool,
            tc.tile_pool(name="sps", bufs=2, space=MS.PSUM) as spool,
            tc.tile_pool(name="o3ps", bufs=1, space=MS.PSUM) as o3pool,
            tc.tile_pool(name="ops", bufs=1, space=MS.PSUM) as opool,
            tc.tile_pool(name="projps", bufs=1, space=MS.PSUM) as projpool,
            tc.tile_pool(name="vtps", bufs=1, space=MS.PSUM) as vtpool,
        ):
            # ---- constants (sync queue; gpsimd queue is for the big loads) ----
            wkv_sb = cpool.tile([128, NT, 128], bf16)
            nc.sync.dma_start(wkv_sb[:], wkv_d[:])
            wq_sb = cpool.tile([128, NT, HD], bf16)
            nc.sync.dma_start(wq_sb[:], wq_d[:])
            ident = cpool.tile([128, 128], f32)
            nc.sync.dma_start(ident[:], ident_d[:])
            identh = cpool.tile([128, 64], bf16)
            nc.sync.dma_start(identh[:], identh_d[:])
            # mask tiles are DMA'd per slot (sync queue) near first use so
            # the 4MB doesn't compete with xt/xq in the startup window
            masks = cpool.tile([128, 16 * 2 * QB], bf16)
            mask_loaded = [False] * 4

            def load_masks(si):
                if not mask_loaded[si]:
                    mask_loaded[si] = True
                    lo, hi = si * 4 * 2 * QB, (si + 1) * 4 * 2 * QB
                    nc.sync.dma_start(masks[:, lo:hi], masks_d[:, lo:hi])

            # ---- persistent sbuf state ----
            kvt = bigpool.tile([128, T], bf16)         # 0:64 K^T, 64:128 V^T
            ktr = bigpool.tile([128, T], bf16)         # 64:128 = K^T replica
            qt_sel = bigpool.tile([128, 4 * QB], bf16) # own Q^T, both halves
            v_aug = bigpool.tile([128, 32 * 65], bf16) # V natural + ones col
            nc.vector.memset(v_aug[:], 1.0)

            o_ps_of = {}

            def emit_epilogue(si):
                o_ps = o_ps_of[si]
                ot_sb = osbpool.tile([65, QB], f32, tag="osb",
                                     name=f"ot{si}")
                nc.any.tensor_copy(ot_sb[:], o_ps[:])
                for u in range(QB // 128):
                    tp_ps = vtpool.tile([128, QB], f32, tag="vt",
                                        name=f"tp{si}_{u}")
                    nc.tensor.transpose(
                        tp_ps[:, 0:65], ot_sb[:, u * 128:(u + 1) * 128],
                        ident[0:65, 0:65])
                    rec = recpool.tile([128, 1], f32, tag="rec",
                                       name=f"rec{si}_{u}")
                    nc.vector.reciprocal(rec[:], tp_ps[:, 64:65])
                    o_nat = onatpool.tile([128, HD], f32, tag="onat",
                                          name=f"onat{si}_{u}")
                    nc.vector.tensor_scalar(
                        o_nat[:], tp_ps[:, 0:HD], rec[:], None,
                        mybir.AluOpType.mult)
                    nc.sync.dma_start(
                        out_d[si * QB + u * 128: si * QB + (u + 1) * 128, :],
                        o_nat[:])

            def emit_S(si, p):
                npair = SLOT_NK[si] // 2
                ka, kb2 = 2 * p, 2 * p + 1
                s_ps = spool.tile([128, 2 * QB], f32, tag="sps",
                                  name=f"s{si}_{p}")
                nc.tensor.matmul(
                    s_ps[:, 0:QB],
                    kvt[0:64, ka * KB:(ka + 1) * KB],
                    qt_sel[0:64, si * QB:(si + 1) * QB],
                    start=True, stop=True)
                nc.tensor.matmul(
                    s_ps[:, QB:2 * QB],
                    ktr[64:128, kb2 * KB:(kb2 + 1) * KB],
                    qt_sel[64:128, si * QB:(si + 1) * QB],
                    start=True, stop=True)
                pt = ptpool.tile([128, 2 * QB], bf16, tag="pt",
                                 name=f"pt{si}_{p}")
                nc.scalar.activation(pt[:], s_ps[:], AF.Exp,
                                     scale=float(HD) ** -0.5)
                m = p - (npair - 4)
                if m >= 0:
                    load_masks(si)
                    idx = (si * 4 + m) * 2 * QB
                    nc.vector.tensor_tensor(
                        pt[:], pt[:], masks[:, idx:idx + 2 * QB],
                        mybir.AluOpType.mult)
                return pt

            def emit_PV(si, p, pt):
                npair = SLOT_NK[si] // 2
                if p == 0:
                    pool = o3pool if si == 3 else opool
                    o_ps_of[si] = pool.tile(
                        [65, QB], f32, name=f"o_ps{si}",
                        tag="o3" if si == 3 else "ops")
                o_ps = o_ps_of[si]
                ka, kb2 = 2 * p, 2 * p + 1
                nc.tensor.matmul(
                    o_ps[:], v_aug[:, ka * 65:ka * 65 + 65], pt[:, 0:QB],
                    start=(p == 0), stop=False, skip_group_check=True)
                nc.tensor.matmul(
                    o_ps[:], v_aug[:, kb2 * 65:kb2 * 65 + 65],
                    pt[:, QB:2 * QB],
                    start=False, stop=(p == npair - 1),
                    skip_group_check=True)
                if p == npair - 1:
                    emit_epilogue(si)

            # software pipeline: PE stream is S(p+1) ... PV(p) so the next S
            # runs during exp(p) and ACT stays saturated
            pend = [None]

            def push_pair(si, p):
                pt = emit_S(si, p)
                if pend[0] is not None:
                    emit_PV(*pend[0])
                pend[0] = (si, p, pt)

            load_masks(0)

            # ---- interleaved input prologue on the gpsimd queue: earliest-
            # needed tensors first, no pool-gated trigger blocks the queue ----
            xq_tiles = []
            xt_tiles = {}

            def xt_load(tb, nchunks):
                xt_sb = xtpool.tile([128, NT, QB], bf16, tag="xt",
                                    name=f"xt{tb}")
                base = tb * NT * QB
                step = NT // nchunks
                for c in range(0, NT, step):
                    nc.gpsimd.dma_start(
                        xt_sb[:, c:c + step, :],
                        xt_d[:, base + c * QB: base + (c + step) * QB])
                xt_tiles[tb] = xt_sb

            def xq_load(si):
                xq_sb = xqpool.tile([128, NT, QB], bf16, tag="xq",
                                    name=f"xq{si}")
                nc.gpsimd.dma_start(xq_sb[:],
                                    xq_d[:, si * NT * QB:(si + 1) * NT * QB])
                xq_tiles.append(xq_sb)

            xq_load(0)
            xt_load(0, 2)
            xq_load(1)
            xt_load(1, 2)
            xq_load(2)
            xq_load(3)
            xt_load(2, 1)

            # ---- up-front Q projections (only need wq + xq) ----
            for si in range(4):
                xq_sb = xq_tiles[si]
                # col-tiled halves write DIAGONAL slices of a 2-bank tile so
                # each accumulation group owns its psum bank outright
                q2_ps = spool.tile([128, 2 * QB], f32, tag="sps",
                                   name=f"q2_{si}")
                for ni in range(NT):
                    nc.tensor.matmul(q2_ps[0:64, 0:QB], wq_sb[:, ni, :],
                                     xq_sb[:, ni, :],
                                     start=(ni == 0), stop=(ni == NT - 1))
                    nc.tensor.matmul(q2_ps[64:128, QB:2 * QB], wq_sb[:, ni, :],
                                     xq_sb[:, ni, :],
                                     start=(ni == 0), stop=(ni == NT - 1))
                nc.vector.tensor_copy(qt_sel[0:64, si * QB:(si + 1) * QB],
                                      q2_ps[0:64, 0:QB])
                nc.vector.tensor_copy(qt_sel[64:128, si * QB:(si + 1) * QB],
                                      q2_ps[64:128, QB:2 * QB])

            # ---- main pipeline over t-blocks ----
            for tb in range(NQB):
                if tb in xt_tiles:
                    xt_sb = xt_tiles[tb]
                else:
                    xt_load(tb, 2)
                    xt_sb = xt_tiles[tb]
                if tb in (2, 4, 5):
                    load_masks({2: 1, 4: 2, 5: 3}[tb])
                kv_ps = projpool.tile([128, QB], f32, tag="proj")
                for ni in range(NT):
                    nc.tensor.matmul(kv_ps[:], wkv_sb[:, ni, :], xt_sb[:, ni, :],
                                     start=(ni == 0), stop=(ni == NT - 1))
                nc.vector.tensor_copy(kvt[:, tb * QB:(tb + 1) * QB], kv_ps[:])
                # replicate K^T to partitions 64:127 for the row-tiled S
                nc.sync.dma_start(ktr[64:128, tb * QB:(tb + 1) * QB],
                                  kvt[0:64, tb * QB:(tb + 1) * QB])
                for j in range(QB // KB):
                    kb = tb * (QB // KB) + j
                    tp_ps = vtpool.tile([128, QB], bf16, tag="vt")
                    nc.tensor.transpose(
                        tp_ps[:, 0:64], kvt[64:128, kb * KB:(kb + 1) * KB],
                        identh[64:128, 0:64])
                    nc.vector.tensor_copy(v_aug[:, kb * 65:kb * 65 + 64],
                                          tp_ps[:, 0:64])
                for si, p in _SCHED.get(tb, []):
                    push_pair(si, p)
            for si, p in _POST:
                push_pair(si, p)
            emit_PV(*pend[0])

    _legalize_matmul_waits(nc)
    return nc


def _legalize_matmul_waits(nc):
    """walrus' LW template encodes at most one sync-wait; hoist extra waits
    from Matmult instructions onto a preceding PE NoOp (same queue, so
    ordering semantics are identical)."""
    import concourse.mybir as mybir

    for f in nc.m.functions:
        for bb in f.blocks:
            new_insts = []
            for inst in bb.instructions:
                si = inst.sync_info
                if (si is not None and si.on_wait and len(si.on_wait) >= 2):
                    for w in si.on_wait:
                        nop = mybir.InstNoOp(
                            name=nc.get_next_instruction_name(),
                            text_hint="wait_hoist", bass_nofuse=True)
                        nop.engine = inst.engine
                        nop.sync_info = mybir.SyncInfo(
                            on_wait=[w], on_update=[])
                        new_insts.append(nop)
                    inst.sync_info = mybir.SyncInfo(
                        on_wait=[], on_update=list(si.on_update or []))
                new_insts.append(inst)
            del bb.instructions[:]
            for i in new_insts:
                bb.instructions.append(i)


def _host_masks(half):
    """16 mask tiles [128, 1024] bf16: slot si, masked pair m covers k-blocks
    kx = nk-8+2m (cols 0:512) and kx+1 (cols 512:1024).
    valid(i, c) iff qoff + c >= kx*128 + i."""
    i = np.arange(128, dtype=np.int32)[:, None]
    c = np.arange(QB, dtype=np.int32)[None, :]
    tiles = []
    for si, nk in enumerate(SLOT_NK):
        qoff = HALF_QBS[half][si] * QB
        for m in range(4):
            for kx in (nk - 8 + 2 * m, nk - 8 + 2 * m + 1):
                tiles.append((qoff + c - kx * 128 - i) >= 0)
    return np.ascontiguousarray(
        np.concatenate(tiles, axis=1).astype(BF16))


def _swizzle(arr):
    """[NE, W] -> [128, (W//QB) * NT * QB] partition-major sbuf layout:
    out[p, (w*NT + nt)*QB + t] = arr[nt*128 + p, w*QB + t]."""
    ne, width = arr.shape
    nw = width // QB
    a = arr.reshape(NT, 128, nw, QB)          # [nt, p, w, t]
    a = a.transpose(1, 2, 0, 3)               # [p, w, nt, t]
    return np.ascontiguousarray(a.reshape(128, nw * NT * QB))


def _swizzle_w(w):
    """[NE, M] -> [128, NT*M]: out[p, nt*M + m] = w[nt*128 + p, m]."""
    m = w.shape[1]
    a = w.reshape(NT, 128, m).transpose(1, 0, 2)
    return np.ascontiguousarray(a.reshape(128, NT * m))


def _make_inputs(x, Wq, Wk, Wv):
    wkv = _swizzle_w(np.concatenate([Wk, Wv], axis=1).astype(BF16))
    wq = _swizzle_w(np.asarray(Wq, dtype=np.float32).astype(BF16))
    ident = np.eye(128, dtype=np.float32)
    identh = np.zeros((128, 64), dtype=np.float32)
    identh[64:128, :] = np.eye(64, dtype=np.float32)
    identh = identh.astype(BF16)

    in_maps = []
    for c in range(8):
        b, half = c // 2, c % 2
        xb = np.asarray(x[b], dtype=np.float32)
        xt = _swizzle(np.ascontiguousarray(xb.T).astype(BF16))
        xq_cols = np.concatenate(
            [xb[qb * QB:(qb + 1) * QB, :].T for qb in HALF_QBS[half]],
            axis=1)
        xq = _swizzle(np.ascontiguousarray(xq_cols).astype(BF16))
        in_maps.append({
            "xt": xt, "xq": xq, "wkv": wkv, "wq": wq, "ident": ident,
            "identh": identh, "masks": _host_masks(half),
        })
    return in_maps


def kernel(x, Wq, Wk, Wv, _want_results=False, _trace=False):
    from concourse import bass_utils

    if "prog" not in _CACHE:
        _CACHE["prog"] = _build_program()
    nc = _CACHE["prog"]
    in_maps = _make_inputs(x, Wq, Wk, Wv)
    res = bass_utils.run_bass_kernel_spmd(nc, in_maps, core_ids=list(range(8)),
                                          trace=_trace)
    out = np.zeros((B, T, HD), dtype=np.float32)
    for c in range(8):
        b, half = c // 2, c % 2
        o = res.results[c]["out"]
        for si in range(4):
            qb = HALF_QBS[half][si]
            out[b, qb * QB:(qb + 1) * QB, :] = o[si * QB:(si + 1) * QB, :]
    if _want_results:
        return out, res
    return out


# revision 20
# speedup vs baseline: 1.9257x; 1.0519x over previous
"""Single-head causal attention (B=4, T=4096, n_embd=1024, head=64) on 8 trn2 cores.

One SPMD program, 8 cores, one launch.  Core c -> batch b=c//2, half h=c%2.
Causal-balanced q-block (512 rows) assignment: half0 {0,3,4,7}, half1 {1,2,5,6}.

Uniform instruction stream across cores; everything core-specific is DATA:
  - xq: the core's own 4 q-blocks of x^T, host-gathered in slot order
  - masks: 16 precomputed [128,1024] 0/1 bf16 tiles (slot si, masked pair m)
All device inputs are host-pre-swizzled into the exact SBUF layout (partition-
major, contiguous free dim) so every input DMA is a plain 2D copy with 8KB
contiguous runs per partition (the naive 3D gather moved only ~1KB per packet
and crawled at ~90 GB/s).

Slot si covers SLOT_NK[si] = {8,16,24,32} k-blocks (128 keys each); the last
4 pairs of each slot are mask-multiplied (covers both the causal diagonal and
the padding when the hosted q-block needs fewer k-blocks than the slot).

Math (S^T formulation, bf16 inputs / fp32 PSUM):
  S^T[tk,tq] = K_blk^T.T @ Q^T   (the 2 k-blocks of a pair run as row-tiled
                                  64x128 matmuls on array rows 0:63 / 64:127
                                  concurrently -> 2 psum banks)
  P^T = exp(S^T / 8) -> bf16     (one ACT op over both banks)
  P^T *= mask                    (DVE tensor_tensor, bf16 2x mode)
  O_aug^T[65,512] += V_aug_blk.T @ P^T   (V_aug col 64 = ones => row 64 of
                                  O_aug accumulates the softmax denominator)

Schedule: all Q projected up front (col-tiled M=64 pairs replicate Q^T to both
partition halves for the row-tiled S); attention pairs are emitted
incrementally one t-block after their keys are projected (slot0@tb1-2,
slot1@tb3-4, slot2@tb5-6, slot3 2 pairs/tb) and SOFTWARE-PIPELINED: the PE
stream is S(p+1) ... PV(p), so the next pair's S runs during exp(p) and the
ACT engine (the steady-state rate limiter) never waits on the serial
exp->mask->PV->S chain.  K^T is replicated to partitions 64:127 via an
SBUF->SBUF DMA with one t-block of slack.
Epilogue per slot (right after its last PV): PE-transpose O_aug^T ->
[128tq,65] fp32, reciprocal of col 64, scale -> natural [128,64] fp32 rows,
DMA out.  Host reassembles slots.
"""

import numpy as np
import ml_dtypes

BF16 = ml_dtypes.bfloat16

B, T, NE, HD = 4, 4096, 1024, 64
QB = 512            # q-block width
KB = 128            # k-block width
NQB = T // QB       # 8 t-blocks
NT = NE // 128      # 8 n-tiles (projection contraction)
SLOT_NK = [8, 16, 24, 32]          # k-blocks per slot (pairs: 4, 8, 12, 16)
HALF_QBS = [[0, 3, 4, 7], [1, 2, 5, 6]]   # slot si hosts q-block HALF_QBS[h][si]

# pair emission schedule: _SCHED[tb] = [(si, p), ...] emitted after proj(tb)
_SCHED = {tb: [] for tb in range(1, NQB)}
for _tb in range(1, 3):
    _SCHED[_tb] += [(0, p) for p in range(2 * (_tb - 1), 2 * _tb)]
for _tb in range(3, 5):
    _SCHED[_tb] += [(1, p) for p in range(4 * (_tb - 3), 4 * (_tb - 2))]
for _tb in range(5, 7):
    _SCHED[_tb] += [(2, p) for p in range(6 * (_tb - 5), 6 * (_tb - 4))]
for _tb in range(1, 8):
    _SCHED[_tb] += [(3, p) for p in range(2 * (_tb - 1), 2 * _tb)]
_POST = [(3, 14), (3, 15)]

_CACHE = {}


def _build_program():
    import concourse.bass as bass
    import concourse.mybir as mybir
    import concourse.tile as tile

    f32 = mybir.dt.float32
    bf16 = mybir.dt.bfloat16
    AF = mybir.ActivationFunctionType
    MS = bass.MemorySpace
    nc = bass.Bass("TRN2", target_bir_lowering=True, debug=False,
                   enable_asserts=False)

    # all pre-swizzled to [128 partitions, contiguous free]
    xt_d = nc.dram_tensor("xt", [128, NQB * NT * QB], bf16,
                          kind="ExternalInput").ap()
    xq_d = nc.dram_tensor("xq", [128, 4 * NT * QB], bf16,
                          kind="ExternalInput").ap()
    wkv_d = nc.dram_tensor("wkv", [128, NT * 128], bf16,
                           kind="ExternalInput").ap()
    wq_d = nc.dram_tensor("wq", [128, NT * HD], bf16,
                          kind="ExternalInput").ap()
    ident_d = nc.dram_tensor("ident", [128, 128], f32, kind="ExternalInput").ap()
    identh_d = nc.dram_tensor("identh", [128, 64], bf16, kind="ExternalInput").ap()
    masks_d = nc.dram_tensor("masks", [128, 16 * 2 * QB], bf16,
                             kind="ExternalInput").ap()
    out_d = nc.dram_tensor("out", [4 * QB, HD], f32, kind="ExternalOutput").ap()

    with tile.TileContext(nc) as tc:
        with (
            tc.tile_pool(name="consts", bufs=1) as cpool,
            tc.tile_pool(name="big", bufs=1) as bigpool,
            tc.tile_pool(name="xt", bufs=3) as xtpool,
            tc.tile_pool(name="xq", bufs=4) as xqpool,
            tc.tile_pool(name="pt", bufs=3) as ptpool,
            tc.tile_pool(name="osb", bufs=4) as osbpool,
            tc.tile_pool(name="onat", bufs=6) as onatpool,
            tc.tile_pool(name="rec", bufs=6) as recpool,
            tc.tile_pool(name="sps", bufs=2, space=MS.PSUM) as spool,
            tc.tile_pool(name="o3ps", bufs=1, space=MS.PSUM) as o3pool,
            tc.tile_pool(name="ops", bufs=1, space=MS.PSUM) as opool,
            tc.tile_pool(name="projps", bufs=1, space=MS.PSUM) as projpool,
            tc.tile_pool(name="vtps", bufs=1, space=MS.PSUM) as vtpool,
        ):
            # ---- constants (sync queue; gpsimd queue is for the big loads) ----
            wkv_sb = cpool.tile([128, NT, 128], bf16)
            nc.sync.dma_start(wkv_sb[:], wkv_d[:])
            wq_sb = cpool.tile([128, NT, HD], bf16)
            nc.sync.dma_start(wq_sb[:], wq_d[:])
            ident = cpool.tile([128, 128], f32)
            nc.sync.dma_start(ident[:], ident_d[:])
            identh = cpool.tile([128, 64], bf16)
            nc.sync.dma_start(identh[:], identh_d[:])
            # mask tiles are DMA'd per slot (sync queue) near first use so
            # the 4MB doesn't compete with xt/xq in the startup window
            masks = cpool.tile([128, 16 * 2 * QB], bf16)
            mask_loaded = [False] * 4

            def load_masks(si):
                if not mask_loaded[si]:
                    mask_loaded[si] = True
                    lo, hi = si * 4 * 2 * QB, (si + 1) * 4 * 2 * QB
                    nc.sync.dma_start(masks[:, lo:hi], masks_d[:, lo:hi])

            # ---- persistent sbuf state ----
            kvt = bigpool.tile([128, T], bf16)         # 0:64 K^T, 64:128 V^T
            ktr = bigpool.tile([128, T], bf16)         # 64:128 = K^T replica
            qt_sel = bigpool.tile([128, 4 * QB], bf16) # own Q^T, both halves
            v_aug = bigpool.tile([128, 32 * 65], bf16) # V natural + ones col
            nc.vector.memset(v_aug[:], 1.0)

            o_ps_of = {}

            def emit_epilogue(si):
                o_ps = o_ps_of[si]
                ot_sb = osbpool.tile([65, QB], f32, tag="osb",
                                     name=f"ot{si}")
                nc.any.tensor_copy(ot_sb[:], o_ps[:])
                for u in range(QB // 128):
                    tp_ps = vtpool.tile([128, QB], f32, tag="vt",
                                        name=f"tp{si}_{u}")
                    nc.tensor.transpose(
                        tp_ps[:, 0:65], ot_sb[:, u * 128:(u + 1) * 128],
                        ident[0:65, 0:65])
                    rec = recpool.tile([128, 1], f32, tag="rec",
                                       name=f"rec{si}_{u}")
                    nc.vector.reciprocal(rec[:], tp_ps[:, 64:65])
                    o_nat = onatpool.tile([128, HD], f32, tag="onat",
                                          name=f"onat{si}_{u}")
                    nc.vector.tensor_scalar(
                        o_nat[:], tp_ps[:, 0:HD], rec[:], None,
                        mybir.AluOpType.mult)
                    nc.sync.dma_start(
                        out_d[si * QB + u * 128: si * QB + (u + 1) * 128, :],
                        o_nat[:])

            def emit_S(si, p):
                npair = SLOT_NK[si] // 2
                ka, kb2 = 2 * p, 2 * p + 1
                s_ps = spool.tile([128, 2 * QB], f32, tag="sps",
                                  name=f"s{si}_{p}")
                nc.tensor.matmul(
                    s_ps[:, 0:QB],
                    kvt[0:64, ka * KB:(ka + 1) * KB],
                    qt_sel[0:64, si * QB:(si + 1) * QB],
                    start=True, stop=True)
                nc.tensor.matmul(
                    s_ps[:, QB:2 * QB],
                    ktr[64:128, kb2 * KB:(kb2 + 1) * KB],
                    qt_sel[64:128, si * QB:(si + 1) * QB],
                    start=True, stop=True)
                pt = ptpool.tile([128, 2 * QB], bf16, tag="pt",
                                 name=f"pt{si}_{p}")
                nc.scalar.activation(pt[:], s_ps[:], AF.Exp,
                                     scale=float(HD) ** -0.5)
                m = p - (npair - 4)
                if m >= 0:
                    load_masks(si)
                    idx = (si * 4 + m) * 2 * QB
                    nc.vector.tensor_tensor(
                        pt[:], pt[:], masks[:, idx:idx + 2 * QB],
                        mybir.AluOpType.mult)
                return pt

            def emit_PV(si, p, pt):
                npair = SLOT_NK[si] // 2
                if p == 0:
                    pool = o3pool if si == 3 else opool
                    o_ps_of[si] = pool.tile(
                        [65, QB], f32, name=f"o_ps{si}",
                        tag="o3" if si == 3 else "ops")
                o_ps = o_ps_of[si]
                ka, kb2 = 2 * p, 2 * p + 1
                nc.tensor.matmul(
                    o_ps[:], v_aug[:, ka * 65:ka * 65 + 65], pt[:, 0:QB],
                    start=(p == 0), stop=False, skip_group_check=True)
                nc.tensor.matmul(
                    o_ps[:], v_aug[:, kb2 * 65:kb2 * 65 + 65],
                    pt[:, QB:2 * QB],
                    start=False, stop=(p == npair - 1),
                    skip_group_check=True)
                if p == npair - 1:
                    emit_epilogue(si)

            # software pipeline: PE stream is S(p+1) ... PV(p) so the next S
            # runs during exp(p) and ACT stays saturated
            pend = [None]

            def push_pair(si, p):
                pt = emit_S(si, p)
                if pend[0] is not None:
                    emit_PV(*pend[0])
                pend[0] = (si, p, pt)

            load_masks(0)

            # ---- interleaved input prologue on the gpsimd queue: earliest-
            # needed tensors first, no pool-gated trigger blocks the queue ----
            xq_tiles = []
            xt_tiles = {}

            def xt_load(tb, nchunks):
                xt_sb = xtpool.tile([128, NT, QB], bf16, tag="xt",
                                    name=f"xt{tb}")
                base = tb * NT * QB
                step = NT // nchunks
                for c in range(0, NT, step):
                    nc.gpsimd.dma_start(
                        xt_sb[:, c:c + step, :],
                        xt_d[:, base + c * QB: base + (c + step) * QB])
                xt_tiles[tb] = xt_sb

            def xq_load(si, eng):
                xq_sb = xqpool.tile([128, NT, QB], bf16, tag="xq",
                                    name=f"xq{si}")
                eng.dma_start(xq_sb[:],
                              xq_d[:, si * NT * QB:(si + 1) * NT * QB])
                xq_tiles.append(xq_sb)

            # three independent trigger queues (gpsimd SWDGE + sync/scalar
            # HWDGE) so the early transfers run concurrently on the shared
            # SDMA engines instead of serializing on one ring
            xq_load(0, nc.sync)
            xq_load(1, nc.scalar)
            xt_load(0, 2)
            xq_load(2, nc.sync)
            xq_load(3, nc.scalar)
            xt_load(1, 2)
            xt2_sb = xtpool.tile([128, NT, QB], bf16, tag="xt", name="xt2")
            nc.scalar.dma_start(xt2_sb[:],
                                xt_d[:, 2 * NT * QB:3 * NT * QB])
            xt_tiles[2] = xt2_sb

            # ---- up-front Q projections (only need wq + xq) ----
            for si in range(4):
                xq_sb = xq_tiles[si]
                # col-tiled halves write DIAGONAL slices of a 2-bank tile so
                # each accumulation group owns its psum bank outright
                q2_ps = spool.tile([128, 2 * QB], f32, tag="sps",
                                   name=f"q2_{si}")
                for ni in range(NT):
                    nc.tensor.matmul(q2_ps[0:64, 0:QB], wq_sb[:, ni, :],
                                     xq_sb[:, ni, :],
                                     start=(ni == 0), stop=(ni == NT - 1))
                    nc.tensor.matmul(q2_ps[64:128, QB:2 * QB], wq_sb[:, ni, :],
                                     xq_sb[:, ni, :],
                                     start=(ni == 0), stop=(ni == NT - 1))
                nc.vector.tensor_copy(qt_sel[0:64, si * QB:(si + 1) * QB],
                                      q2_ps[0:64, 0:QB])
                nc.vector.tensor_copy(qt_sel[64:128, si * QB:(si + 1) * QB],
                                      q2_ps[64:128, QB:2 * QB])

            # ---- main pipeline over t-blocks ----
            for tb in range(NQB):
                if tb in xt_tiles:
                    xt_sb = xt_tiles[tb]
                else:
                    xt_load(tb, 2)
                    xt_sb = xt_tiles[tb]
                if tb in (2, 4, 5):
                    load_masks({2: 1, 4: 2, 5: 3}[tb])
                sched = _SCHED.get(tb, [])
                for si, p in sched[:2]:
                    push_pair(si, p)
                kv_ps = projpool.tile([128, QB], f32, tag="proj")
                for ni in range(NT):
                    nc.tensor.matmul(kv_ps[:], wkv_sb[:, ni, :], xt_sb[:, ni, :],
                                     start=(ni == 0), stop=(ni == NT - 1))
                nc.vector.tensor_copy(kvt[:, tb * QB:(tb + 1) * QB], kv_ps[:])
                # replicate K^T to partitions 64:127 for the row-tiled S
                nc.sync.dma_start(ktr[64:128, tb * QB:(tb + 1) * QB],
                                  kvt[0:64, tb * QB:(tb + 1) * QB])
                for j in range(QB // KB):
                    kb = tb * (QB // KB) + j
                    tp_ps = vtpool.tile([128, QB], bf16, tag="vt")
                    nc.tensor.transpose(
                        tp_ps[:, 0:64], kvt[64:128, kb * KB:(kb + 1) * KB],
                        identh[64:128, 0:64])
                    nc.vector.tensor_copy(v_aug[:, kb * 65:kb * 65 + 64],
                                          tp_ps[:, 0:64])
                for si, p in sched[2:]:
                    push_pair(si, p)
            for si, p in _POST:
                push_pair(si, p)
            emit_PV(*pend[0])

    _legalize_matmul_waits(nc)
    return nc


def _legalize_matmul_waits(nc):
    """walrus' LW template encodes at most one sync-wait; hoist extra waits
    from Matmult instructions onto a preceding PE NoOp (same queue, so
    ordering semantics are identical)."""
    import concourse.mybir as mybir

    for f in nc.m.functions:
        for bb in f.blocks:
            new_insts = []
            for inst in bb.instructions:
                si = inst.sync_info
                if (si is not None and si.on_wait and len(si.on_wait) >= 2):
                    for w in si.on_wait:
                        nop = mybir.InstNoOp(
                            name=nc.get_next_instruction_name(),
                            text_hint="wait_hoist", bass_nofuse=True)
                        nop.engine = inst.engine
                        nop.sync_info = mybir.SyncInfo(
                            on_wait=[w], on_update=[])
                        new_insts.append(nop)
                    inst.sync_info = mybir.SyncInfo(
                        on_wait=[], on_update=list(si.on_update or []))
                new_insts.append(inst)
            del bb.instructions[:]
            for i in new_insts:
                bb.instructions.append(i)


def _host_masks(half):
    """16 mask tiles [128, 1024] bf16: slot si, masked pair m covers k-blocks
    kx = nk-8+2m (cols 0:512) and kx+1 (cols 512:1024).
    valid(i, c) iff qoff + c >= kx*128 + i."""
    i = np.arange(128, dtype=np.int32)[:, None]
    c = np.arange(QB, dtype=np.int32)[None, :]
    tiles = []
    for si, nk in enumerate(SLOT_NK):
        qoff = HALF_QBS[half][si] * QB
        for m in range(4):
            for kx in (nk - 8 + 2 * m, nk - 8 + 2 * m + 1):
                tiles.append((qoff + c - kx * 128 - i) >= 0)
    return np.ascontiguousarray(
        np.concatenate(tiles, axis=1).astype(BF16))


def _swizzle(arr):
    """[NE, W] -> [128, (W//QB) * NT * QB] partition-major sbuf layout:
    out[p, (w*NT + nt)*QB + t] = arr[nt*128 + p, w*QB + t]."""
    ne, width = arr.shape
    nw = width // QB
    a = arr.reshape(NT, 128, nw, QB)          # [nt, p, w, t]
    a = a.transpose(1, 2, 0, 3)               # [p, w, nt, t]
    return np.ascontiguousarray(a.reshape(128, nw * NT * QB))


def _swizzle_w(w):
    """[NE, M] -> [128, NT*M]: out[p, nt*M + m] = w[nt*128 + p, m]."""
    m = w.shape[1]
    a = w.reshape(NT, 128, m).transpose(1, 0, 2)
    return np.ascontiguousarray(a.reshape(128, NT * m))


def _make_inputs(x, Wq, Wk, Wv):
    wkv = _swizzle_w(np.concatenate([Wk, Wv], axis=1).astype(BF16))
    wq = _swizzle_w(np.asarray(Wq, dtype=np.float32).astype(BF16))
    ident = np.eye(128, dtype=np.float32)
    identh = np.zeros((128, 64), dtype=np.float32)
    identh[64:128, :] = np.eye(64, dtype=np.float32)
    identh = identh.astype(BF16)

    in_maps = []
    for c in range(8):
        b, half = c // 2, c % 2
        xb = np.asarray(x[b], dtype=np.float32)
        xt = _swizzle(np.ascontiguousarray(xb.T).astype(BF16))
        xq_cols = np.concatenate(
            [xb[qb * QB:(qb + 1) * QB, :].T for qb in HALF_QBS[half]],
            axis=1)
        xq = _swizzle(np.ascontiguousarray(xq_cols).astype(BF16))
        in_maps.append({
            "xt": xt, "xq": xq, "wkv": wkv, "wq": wq, "ident": ident,
            "identh": identh, "masks": _host_masks(half),
        })
    return in_maps


def kernel(x, Wq, Wk, Wv, _want_results=False, _trace=False):
    from concourse import bass_utils

    if "prog" not in _CACHE:
        _CACHE["prog"] = _build_program()
    nc = _CACHE["prog"]
    in_maps = _make_inputs(x, Wq, Wk, Wv)
    res = bass_utils.run_bass_kernel_spmd(nc, in_maps, core_ids=list(range(8)),
                                          trace=_trace)
    out = np.zeros((B, T, HD), dtype=np.float32)
    for c in range(8):
        b, half = c // 2, c % 2
        o = res.results[c]["out"]
        for si in range(4):
            qb = HALF_QBS[half][si]
            out[b, qb * QB:(qb + 1) * QB, :] = o[si * QB:(si + 1) * QB, :]
    if _want_results:
        return out, res
    return out
